# revision 1
# baseline (speedup 1.0000x reference)
"""Trainium2 Bass kernel for nn_Cate1Classifier (SWEM title/desc pooling +
FC + BatchNorm(train) + ReLU + classifier), data-parallel over 8 NeuronCores.

Contract: kernel(**inputs) takes the FULL unsharded inputs (as produced by
setup_inputs()) and returns the FULL [1024, 10] float32 output.

Design notes:
- Batch (1024) is sharded 128/core across 8 cores; embedding table and all
  weights are replicated.
- Embedding gather: per-token-position indirect DMAs ([128, 512] f32 each,
  one offset per partition — the only form the HW DGE supports).
- Padding is handled index-side: padded slots gather a duplicate of the
  sample's token 0. Max-pool is then exact with no masking; sum-pool is
  fixed up with one fused multiply-add (acc -= npad * e_tok0).
- Pooled accumulators are PE-transposed so the FC produces h^T
  (hidden-on-partitions); BatchNorm scale/shift become per-partition
  scalars applied by the ACT engine fused with ReLU.
- BatchNorm uses full-batch statistics: per-core sum(h), sum(h^2) are
  AllReduce'd across the 8 cores (8KB payload).
- b_fc is omitted: BN immediately follows the FC, so a constant column
  shift cancels exactly in (h - mean).
"""

import sys

for _p in ("/opt/trn_rl_repo", "/root/.axon_site/_ro/trn_rl_repo"):
    if _p not in sys.path:
        sys.path.insert(0, _p)

import numpy as np

from concourse import bass, bacc, tile, mybir
from concourse import bass_utils

# Problem shape (hardcoded per the task contract).
B, LT, LD = 1024, 50, 200
V, D = 100000, 512
H, C = 1024, 10
N_CORES = 8
PB = B // N_CORES  # 128 samples per core
KC = 25  # token positions per gather/reduce chunk
BN_EPS = 1e-5

F32 = mybir.dt.float32
I32 = mybir.dt.int32
AF = mybir.ActivationFunctionType
OP = mybir.AluOpType

_PROGRAM = None


def _tree_reduce(nc, g, s, acc, op, first_chunk):
    """Reduce the 25 [128, D] slices of chunk tile g with `op` into acc.

    First level folds into scratch s so g is preserved (the PE sum-matmuls
    read g concurrently).
    """
    ts = nc.vector.tensor_tensor
    ts(out=s[:, 0:12 * D], in0=g[:, 0:12 * D], in1=g[:, 12 * D:24 * D], op=op)
    for a, b, n in ((0, 6, 6), (0, 3, 3), (1, 2, 1), (0, 1, 1)):
        ts(out=s[:, a * D:(a + n) * D], in0=s[:, a * D:(a + n) * D],
           in1=s[:, b * D:(b + n) * D], op=op)
    if first_chunk:
        ts(out=acc[:], in0=s[:, 0:D], in1=g[:, 24 * D:25 * D], op=op)
    else:
        ts(out=s[:, 0:D], in0=s[:, 0:D], in1=g[:, 24 * D:25 * D], op=op)
        ts(out=acc[:], in0=acc[:], in1=s[:, 0:D], op=op)


def _build():
    nc = bacc.Bacc("TRN2", target_bir_lowering=False, debug=False,
                   num_devices=N_CORES)

    t_idx = nc.dram_tensor("t_idx", [PB, LT], I32, kind="ExternalInput")
    d_idx = nc.dram_tensor("d_idx", [PB, LD], I32, kind="ExternalInput")
    scal = nc.dram_tensor("scal", [PB, 4], F32, kind="ExternalInput")
    emb = nc.dram_tensor("emb", [V, D], F32, kind="ExternalInput")
    wfc = nc.dram_tensor("wfc", [4 * D, H], F32, kind="ExternalInput")
    wclf = nc.dram_tensor("wclf", [H, C], F32, kind="ExternalInput")
    bclf = nc.dram_tensor("bclf", [1, C], F32, kind="ExternalInput")
    gamma_t = nc.dram_tensor("gamma_t", [128, 8], F32, kind="ExternalInput")
    beta_t = nc.dram_tensor("beta_t", [128, 8], F32, kind="ExternalInput")
    ident = nc.dram_tensor("ident", [128, 128], F32, kind="ExternalInput")
    ones1 = nc.dram_tensor("ones1", [1, 128], F32, kind="ExternalInput")
    # per-chunk diag(-npad_chunk) matrices (title 2 + desc 8, stacked)
    dnpad = nc.dram_tensor("dnpad", [10 * 128, 128], F32, kind="ExternalInput")
    logits = nc.dram_tensor("logits", [PB, C], F32, kind="ExternalOutput")

    with tile.TileContext(nc) as tc:
        with tc.tile_pool(name="const", bufs=1) as cp, \
             tc.tile_pool(name="gpool", bufs=2) as gp, \
             tc.tile_pool(name="spool", bufs=1) as sp, \
             tc.tile_pool(name="wpool", bufs=5) as wp, \
             tc.tile_pool(name="psA", bufs=2, space="PSUM") as psA, \
             tc.tile_pool(name="psB", bufs=1, space="PSUM") as psB, \
             tc.tile_pool(name="psS", bufs=1, space="PSUM") as psS, \
             tc.tile_pool(name="dram", bufs=1, space="DRAM") as dp:

            # --- constant loads ---
            t_idx_t = cp.tile([PB, LT], I32, tag="tidx")
            d_idx_t = cp.tile([PB, LD], I32, tag="didx")
            scal_t = cp.tile([PB, 4], F32, tag="scal")
            gam_t = cp.tile([128, 8], F32, tag="gam")
            bet_t = cp.tile([128, 8], F32, tag="bet")
            id_t = cp.tile([128, 128], F32, tag="ident")
            on_t = cp.tile([1, 128], F32, tag="ones1")
            bc_t = cp.tile([1, C], F32, tag="bclf")
            for dst, src in ((t_idx_t, t_idx), (d_idx_t, d_idx), (scal_t, scal),
                             (gam_t, gamma_t), (bet_t, beta_t), (id_t, ident),
                             (on_t, ones1), (bc_t, bclf)):
                nc.sync.dma_start(dst[:], src[:])
            wclf_t = []
            for mb in range(8):
                w = cp.tile([128, C], F32, tag=f"wclf{mb}")
                nc.sync.dma_start(w[:], wclf[mb * 128:(mb + 1) * 128, :])
                wclf_t.append(w)
            dnp_t = []
            for i in range(10):
                dt_ = cp.tile([128, 128], F32, tag=f"dnp{i}", name=f"dnp{i}")
                nc.sync.dma_start(dt_[:], dnpad[i * 128:(i + 1) * 128, :])
                dnp_t.append(dt_)

            # --- pooling: acc tiles + gather/reduce chunks ---
            accs = {}
            saved = {}
            chunk_base = {"t": 0, "d": 2}
            for fld, idx_t, L, inv_col in (
                    ("t", t_idx_t, LT, 0), ("d", d_idx_t, LD, 1)):
                acc_s = cp.tile([PB, D], F32, tag=f"acc_s{fld}", name=f"acc_s{fld}")
                acc_m = cp.tile([PB, D], F32, tag=f"acc_m{fld}", name=f"acc_m{fld}")
                sav = cp.tile([PB, D], F32, tag=f"sav{fld}", name=f"sav{fld}")
                ps_s = psS.tile([128, D], F32, tag=f"ps_s{fld}", name=f"ps_s{fld}")
                accs[fld] = (acc_s, acc_m)
                saved[fld] = sav
                nchunks = L // KC
                for c in range(nchunks):
                    g = gp.tile([PB, KC * D], F32, tag="g")
                    for j in range(KC):
                        p = c * KC + j
                        nc.gpsimd.indirect_dma_start(
                            out=g[:, j * D:(j + 1) * D], out_offset=None,
                            in_=emb[:],
                            in_offset=bass.IndirectOffsetOnAxis(
                                ap=idx_t[:, p:p + 1], axis=0),
                        )
                    if c == 0:
                        nc.vector.tensor_copy(sav[:], g[:, 0:D])
                    # sum-pool on the (otherwise idle) PE: psum += I.T @ e_p
                    for j in range(KC):
                        p = c * KC + j
                        nc.tensor.matmul(ps_s[:], lhsT=id_t[:],
                                         rhs=g[:, j * D:(j + 1) * D],
                                         start=(p == 0), stop=False)
                    # cancel this chunk's padding (padded slots duplicate
                    # token 0): psum += diag(-npad_chunk) @ e_tok0. Keeping
                    # this per-chunk bounds the f32 partial-sum magnitude.
                    nc.tensor.matmul(ps_s[:], lhsT=dnp_t[chunk_base[fld] + c][:],
                                     rhs=sav[:],
                                     start=False, stop=(c == nchunks - 1))
                    # max-pool tree on DVE
                    s = sp.tile([PB, 12 * D], F32, tag="scr")
                    _tree_reduce(nc, g, s, acc_m, OP.max, c == 0)
                # avg = psum_sum / len
                nc.vector.tensor_scalar_mul(
                    acc_s[:], ps_s[:], scal_t[:, inv_col:inv_col + 1])

            # --- transpose pooled features: swem^T, 16 [128,128] tiles ---
            # swem column order: [t_avg | t_max | d_avg | d_max]
            order = [accs["t"][0], accs["t"][1], accs["d"][0], accs["d"][1]]
            swemT = []
            for i in range(16):
                src = order[i // 4]
                blk = i % 4
                pt = psA.tile([128, 128], F32, tag="tps")
                nc.tensor.transpose(pt[:], src[:, blk * 128:(blk + 1) * 128], id_t[:])
                st = cp.tile([128, 128], F32, tag=f"swemT{i}")
                nc.vector.tensor_copy(st[:], pt[:])
                swemT.append(st)

            # --- FC: h^T[mb] [128 hidden, 128 samples], mb in 0..7 ---
            h_ps = [psB.tile([128, 512], F32, tag="hps0", name="hps0"),
                    psB.tile([128, 512], F32, tag="hps1", name="hps1")]
            # PSUM `start` clears the has_written bits for the WHOLE bank, so
            # emit start only on the first matmul touching each bank (the
            # other slices then overwrite-on-first-touch per element), and
            # stop only on the last matmul into that bank.
            for kc in range(16):
                w = wp.tile([128, H], F32, tag="wfc")
                nc.sync.dma_start(w[:], wfc[kc * 128:(kc + 1) * 128, :])
                for mb in range(8):
                    nc.tensor.matmul(
                        h_ps[mb // 4][:, (mb % 4) * 128:(mb % 4 + 1) * 128],
                        lhsT=w[:, mb * 128:(mb + 1) * 128],
                        rhs=swemT[kc][:],
                        start=(kc == 0 and mb % 4 == 0),
                        stop=(kc == 15 and mb % 4 == 3))

            # --- batch stats: s1 = sum_n h, s2 = sum_n h^2 (per hidden) ---
            # read h straight from PSUM; square on the idle ACT engine
            s12 = cp.tile([128, 16], F32, tag="s12")
            for mb in range(8):
                hps = h_ps[mb // 4][:, (mb % 4) * 128:(mb % 4 + 1) * 128]
                sq = sp.tile([128, 128], F32, tag="sq")
                nc.scalar.activation(sq[:], hps, AF.Square)
                nc.vector.reduce_sum(s12[:, mb:mb + 1], hps, axis=mybir.AxisListType.X)
                nc.vector.reduce_sum(s12[:, 8 + mb:9 + mb], sq[:], axis=mybir.AxisListType.X)

            # --- AllReduce batch stats across the 8 cores ---
            import os
            cc_in = dp.tile([128, 16], F32, tag="ccin")
            cc_out = dp.tile([128, 16], F32, tag="ccout")
            nc.sync.dma_start(cc_in[:], s12[:])
            if os.environ.get("KERNEL_NO_CC"):
                # timing-study variant (TimelineSim can't model collectives);
                # numerically WRONG across cores — never use for grading
                nc.sync.dma_start(cc_out[:], cc_in[:])
            else:
                nc.gpsimd.collective_compute(
                    "AllReduce", OP.add,
                    replica_groups=[list(range(N_CORES))],
                    ins=[cc_in.opt()], outs=[cc_out.opt()],
                )
            s12g = cp.tile([128, 16], F32, tag="s12g")
            nc.sync.dma_start(s12g[:], cc_out[:])

            # --- BN scale/shift (per hidden unit, [128, 8]) ---
            mean = cp.tile([128, 8], F32, tag="mean")
            var = cp.tile([128, 8], F32, tag="var")
            scale = cp.tile([128, 8], F32, tag="scale")
            shift = cp.tile([128, 8], F32, tag="shift")
            inv_b = 1.0 / float(B)
            nc.vector.tensor_scalar_mul(mean[:], s12g[:, 0:8], inv_b)
            nc.vector.tensor_mul(scale[:], mean[:], mean[:])        # scale = mean^2 (tmp)
            nc.vector.tensor_scalar_mul(var[:], s12g[:, 8:16], inv_b)  # var = E[h^2]
            nc.vector.tensor_sub(var[:], var[:], scale[:])          # var -= mean^2
            nc.vector.tensor_scalar_add(var[:], var[:], BN_EPS)
            nc.scalar.activation(var[:], var[:], AF.Sqrt)           # var = std
            nc.vector.reciprocal(scale[:], var[:])                  # scale = 1/std
            nc.vector.tensor_mul(scale[:], scale[:], gam_t[:])      # scale *= gamma
            nc.vector.tensor_mul(shift[:], mean[:], scale[:])       # shift = mean*scale
            nc.vector.tensor_sub(shift[:], bet_t[:], shift[:])      # shift = beta - mean*scale

            # --- BN apply + ReLU on ACT, then classifier matmuls ---
            o_ps = psB.tile([128, C], F32, tag="ops")
            for mb in range(8):
                r = cp.tile([128, 128], F32, tag=f"rT{mb}", name=f"rT{mb}")
                nc.scalar.activation(r[:], h_ps[mb // 4][:, (mb % 4) * 128:(mb % 4 + 1) * 128],
                                     AF.Relu,
                                     bias=shift[:, mb:mb + 1],
                                     scale=scale[:, mb:mb + 1])
                nc.tensor.matmul(o_ps[:], lhsT=r[:], rhs=wclf_t[mb][:],
                                 start=(mb == 0), stop=False)
            # + b_clf via rank-1 ones matmul
            nc.tensor.matmul(o_ps[:], lhsT=on_t[:], rhs=bc_t[:],
                             start=False, stop=True)
            out_sb = cp.tile([128, C], F32, tag="outsb")
            nc.vector.tensor_copy(out_sb[:], o_ps[:])
            nc.sync.dma_start(logits[:], out_sb[:])

    nc.compile()
    return nc


def _get_program():
    global _PROGRAM
    if _PROGRAM is None:
        _PROGRAM = _build()
    return _PROGRAM


def _prep_in_maps(title, desc, t_len, d_len, emb, W_fc, b_fc, gamma, beta,
                  W_clf, b_clf):
    # sanitize: padded positions duplicate token 0 (keeps max exact; sum is
    # corrected on-device with the npad counts)
    def sanitize(tok, lens, L):
        tok = np.asarray(tok)
        valid = np.arange(L)[None, :] < np.asarray(lens)[:, None]
        return np.where(valid, tok, tok[:, :1]).astype(np.int32)

    title_s = sanitize(title, t_len, LT)
    desc_s = sanitize(desc, d_len, LD)
    t_len = np.asarray(t_len).astype(np.float64)
    d_len = np.asarray(d_len).astype(np.float64)
    scal = np.stack([
        1.0 / np.maximum(t_len, 1.0),
        1.0 / np.maximum(d_len, 1.0),
        -(LT - t_len),
        -(LD - d_len),
    ], axis=1).astype(np.float32)  # [B, 4]

    emb = np.ascontiguousarray(np.asarray(emb, dtype=np.float32))
    wfc = np.ascontiguousarray(np.asarray(W_fc, dtype=np.float32))
    wclf = np.ascontiguousarray(np.asarray(W_clf, dtype=np.float32))
    bclf = np.asarray(b_clf, dtype=np.float32).reshape(1, C)
    gamma_t = np.ascontiguousarray(
        np.asarray(gamma, dtype=np.float32).reshape(8, 128).T)
    beta_t = np.ascontiguousarray(
        np.asarray(beta, dtype=np.float32).reshape(8, 128).T)
    ident = np.eye(128, dtype=np.float32)
    ones1 = np.ones((1, 128), dtype=np.float32)

    # diag(-npad_chunk) per (field, chunk): npad_chunk[p] = # padded slots of
    # sample p among positions [c*KC, (c+1)*KC)
    def pad_counts(lens, L):
        cs = []
        for c in range(L // KC):
            a, b = c * KC, (c + 1) * KC
            cs.append(np.clip(b - np.maximum(lens, a), 0, KC))
        return cs  # list of [B]

    npc = pad_counts(t_len, LT) + pad_counts(d_len, LD)  # 2 + 8 chunks
    rng128 = np.arange(128)

    in_maps = []
    for i in range(N_CORES):
        sl = slice(i * PB, (i + 1) * PB)
        dn = np.zeros((10 * 128, 128), dtype=np.float32)
        for ci, cnt in enumerate(npc):
            dn[ci * 128 + rng128, rng128] = -cnt[sl].astype(np.float32)
        in_maps.append({
            "t_idx": np.ascontiguousarray(title_s[sl]),
            "d_idx": np.ascontiguousarray(desc_s[sl]),
            "scal": np.ascontiguousarray(scal[sl]),
            "emb": emb, "wfc": wfc, "wclf": wclf, "bclf": bclf,
            "gamma_t": gamma_t, "beta_t": beta_t,
            "ident": ident, "ones1": ones1, "dnpad": dn,
        })
    return in_maps


def kernel(title, desc, t_len, d_len, emb, W_fc, b_fc, gamma, beta,
           W_clf, b_clf):
    nc = _get_program()
    in_maps = _prep_in_maps(title, desc, t_len, d_len, emb, W_fc, b_fc,
                            gamma, beta, W_clf, b_clf)
    res = bass_utils.run_bass_kernel_spmd(nc, in_maps,
                                          core_ids=list(range(N_CORES)))
    return np.concatenate([res.results[i]["logits"] for i in range(N_CORES)],
                          axis=0)



# revision 4
# speedup vs baseline: 2.0798x; 2.0798x over previous
"""Trainium2 Bass kernel for nn_Cate1Classifier (SWEM title/desc pooling +
FC + BatchNorm(train) + ReLU + classifier), data-parallel over 8 NeuronCores.

Contract: kernel(**inputs) takes the FULL unsharded inputs (as produced by
setup_inputs()) and returns the FULL [1024, 10] float32 output.

v3 design (v1 postmortem: PE 265us fp32 matmuls, Pool 259us serialized
1-index gathers, DMA 208us fp32 traffic were the three near-critical
resources):
- bf16 embedding/weights (host-cast): halves HBM gather traffic and runs PE
  matmuls at 1 cycle/row instead of 4.
- Embedding gather via gpsimd.dma_gather (InstDMAGatherAnt): batches up to
  1024 indices per instruction (vs 1 index/partition for plain indirect
  DMA), amortizing the ~1us SWDGE fixed cost 8x. Indices are int16, so each
  core gets a host-compacted private vocabulary (~27.4k unique tokens of
  the 32k it touches; 32768 rows statically) uploaded as its `emb` input.
  Index layout: flat j-major (token (p,j) at flat j*128+p), wrapped over 16
  partitions and replicated to all 128 (HW reads the replicas).
- Padding handled index-side: an invalid position duplicates the row's
  chunk-start token (or token 0 if the whole chunk is past the row's
  length). Max-pool is then exact with no masking; sum-pool is fixed per
  chunk by one diag(-npad) matmul whose rhs is the chunk's own column 0.
- Sum-pool rides the PE as identity matmuls accumulating in PSUM (f32);
  max-pool is a bf16 DVE tensor_tensor tree.
- Pooled accumulators are PE-transposed so the FC produces h^T
  (hidden-on-partitions); BatchNorm scale/shift become per-partition
  scalars applied by the ACT engine fused with ReLU.
- BatchNorm uses full-batch statistics: per-core sum(h), sum(h^2) are
  AllReduce'd across the 8 cores (8KB payload).
- b_fc is omitted: BN immediately follows the FC, so a constant column
  shift cancels exactly in (h - mean).
"""

import sys

for _p in ("/opt/trn_rl_repo", "/root/.axon_site/_ro/trn_rl_repo"):
    if _p not in sys.path:
        sys.path.insert(0, _p)

import numpy as np
import ml_dtypes

from concourse import bass, bacc, tile, mybir
from concourse import bass_utils

# Problem shape (hardcoded per the task contract).
B, LT, LD = 1024, 50, 200
V, D = 100000, 512
H, C = 1024, 10
N_CORES = 8
PB = B // N_CORES   # 128 samples per core
KC = 25             # positions per reduce chunk
NT, ND = LT // KC, LD // KC
GW = (8, 8, 8, 1)   # gather widths per chunk (dma_gather caps at 1024 idxs)
U = 32768           # compact per-core vocab rows (int16-addressable)
NPOS = LT + LD      # 250
BN_EPS = 1e-5

F32 = mybir.dt.float32
BF16 = mybir.dt.bfloat16
I16 = mybir.dt.int16
AF = mybir.ActivationFunctionType
OP = mybir.AluOpType
BF_NP = np.dtype(ml_dtypes.bfloat16)

_PROGRAM = None


def _tree_reduce(nc, g, s, acc, op, first_chunk):
    """Max-reduce the 25 [128, D] position slices of chunk tile g into acc.

    First level folds into scratch s so g is preserved (the PE sum-matmuls
    read g concurrently).
    """
    ts = nc.vector.tensor_tensor
    ts(out=s[:, 0:12 * D], in0=g[:, 0:12 * D], in1=g[:, 12 * D:24 * D], op=op)
    for a, b_, n in ((0, 6, 6), (0, 3, 3), (1, 2, 1), (0, 1, 1)):
        ts(out=s[:, a * D:(a + n) * D], in0=s[:, a * D:(a + n) * D],
           in1=s[:, b_ * D:(b_ + n) * D], op=op)
    if first_chunk:
        ts(out=acc[:], in0=s[:, 0:D], in1=g[:, 24 * D:25 * D], op=op)
    else:
        ts(out=s[:, 0:D], in0=s[:, 0:D], in1=g[:, 24 * D:25 * D], op=op)
        ts(out=acc[:], in0=acc[:], in1=s[:, 0:D], op=op)


def _build():
    nc = bacc.Bacc("TRN2", target_bir_lowering=False, debug=False,
                   num_devices=N_CORES)

    # idx: per (chunk, gather) blocks, each 16-partition-wrapped flat j-major
    idx = nc.dram_tensor("idx", [128, NPOS * 8], I16, kind="ExternalInput")
    scal = nc.dram_tensor("scal", [PB, 2], F32, kind="ExternalInput")
    emb = nc.dram_tensor("emb", [U, D], BF16, kind="ExternalInput")
    wfc = nc.dram_tensor("wfc", [4 * D, H], BF16, kind="ExternalInput")
    wclf = nc.dram_tensor("wclf", [H, C], BF16, kind="ExternalInput")
    bclf = nc.dram_tensor("bclf", [1, C], BF16, kind="ExternalInput")
    gamma_t = nc.dram_tensor("gamma_t", [128, 8], F32, kind="ExternalInput")
    beta_t = nc.dram_tensor("beta_t", [128, 8], F32, kind="ExternalInput")
    ident = nc.dram_tensor("ident", [128, 128], BF16, kind="ExternalInput")
    ones1 = nc.dram_tensor("ones1", [1, 128], BF16, kind="ExternalInput")
    # per-chunk diag(-npad_chunk) matrices (title 2 + desc 8, stacked)
    dnpad = nc.dram_tensor("dnpad", [(NT + ND) * 128, 128], BF16,
                           kind="ExternalInput")
    logits = nc.dram_tensor("logits", [PB, C], F32, kind="ExternalOutput")

    with tile.TileContext(nc) as tc:
        with tc.tile_pool(name="const", bufs=1) as cp, \
             tc.tile_pool(name="gpool", bufs=2) as gp, \
             tc.tile_pool(name="spool", bufs=1) as sp, \
             tc.tile_pool(name="wpool", bufs=5) as wp, \
             tc.tile_pool(name="psA", bufs=2, space="PSUM") as psA, \
             tc.tile_pool(name="psB", bufs=1, space="PSUM") as psB, \
             tc.tile_pool(name="psS", bufs=1, space="PSUM") as psS, \
             tc.tile_pool(name="dram", bufs=1, space="DRAM") as dp:

            # --- constant loads ---
            idx_t = cp.tile([128, NPOS * 8], I16, tag="idx")
            scal_t = cp.tile([PB, 2], F32, tag="scal")
            gam_t = cp.tile([128, 8], F32, tag="gam")
            bet_t = cp.tile([128, 8], F32, tag="bet")
            id_t = cp.tile([128, 128], BF16, tag="ident")
            on_t = cp.tile([1, 128], BF16, tag="ones1")
            bc_t = cp.tile([1, C], BF16, tag="bclf")
            for dst, src in ((idx_t, idx), (scal_t, scal),
                             (gam_t, gamma_t), (bet_t, beta_t), (id_t, ident),
                             (on_t, ones1), (bc_t, bclf)):
                nc.sync.dma_start(dst[:], src[:])
            wclf_t = []
            for mb in range(8):
                w = cp.tile([128, C], BF16, tag=f"wclf{mb}")
                nc.sync.dma_start(w[:], wclf[mb * 128:(mb + 1) * 128, :])
                wclf_t.append(w)
            dnp_t = []
            for i in range(NT + ND):
                dt_ = cp.tile([128, 128], BF16, tag=f"dnp{i}", name=f"dnp{i}")
                nc.sync.dma_start(dt_[:], dnpad[i * 128:(i + 1) * 128, :])
                dnp_t.append(dt_)

            # --- pooling ---
            acc = {}
            ps_sum = {}
            for fld in ("t", "d"):
                acc[fld] = (
                    cp.tile([PB, D], BF16, tag=f"acc_s{fld}", name=f"acc_s{fld}"),
                    cp.tile([PB, D], BF16, tag=f"acc_m{fld}", name=f"acc_m{fld}"),
                )
                ps_sum[fld] = psS.tile([128, D], F32, tag=f"ps_s{fld}",
                                       name=f"ps_s{fld}")
            chunks = [("t", c, c) for c in range(NT)] + \
                     [("d", c, NT + c) for c in range(ND)]
            nchunks = {"t": NT, "d": ND}
            for fld, c, ci in chunks:
                ps_s = ps_sum[fld]
                g = gp.tile([PB, KC * D], BF16, tag="g")
                pos0 = ci * KC
                off = 0
                for w in GW:
                    n = w * 128
                    nc.gpsimd.dma_gather(
                        out_ap=g[:, off * D:(off + w) * D].rearrange(
                            "p (k d) -> p k d", d=D),
                        in_ap=emb[:],
                        idxs_ap=idx_t[:, (pos0 + off) * 8:(pos0 + off + w) * 8],
                        num_idxs=n, num_idxs_reg=n, elem_size=D)
                    off += w
                # sum-pool on the PE: psum += I.T @ e_p
                for j in range(KC):
                    nc.tensor.matmul(ps_s[:], lhsT=id_t[:],
                                     rhs=g[:, j * D:(j + 1) * D],
                                     start=(c == 0 and j == 0), stop=False)
                # cancel this chunk's padding: invalid slots duplicate the
                # row's own column-0 value, so psum += diag(-npad) @ g[:,0:D]
                # undoes them exactly; per-chunk keeps f32 partials bounded.
                nc.tensor.matmul(ps_s[:], lhsT=dnp_t[ci][:],
                                 rhs=g[:, 0:D],
                                 start=False, stop=(c == nchunks[fld] - 1))
                # max-pool tree on DVE
                s = sp.tile([PB, 12 * D], BF16, tag="scr")
                _tree_reduce(nc, g, s, acc[fld][1], OP.max, c == 0)
            # avg = psum_sum / len
            for fld, inv_col in (("t", 0), ("d", 1)):
                nc.vector.tensor_scalar_mul(
                    acc[fld][0][:], ps_sum[fld][:],
                    scal_t[:, inv_col:inv_col + 1])

            # --- transpose pooled features: swem^T, 16 [128,128] tiles ---
            # swem column order: [t_avg | t_max | d_avg | d_max]
            order = [acc["t"][0], acc["t"][1], acc["d"][0], acc["d"][1]]
            swemT = []
            for i in range(16):
                src = order[i // 4]
                blk = i % 4
                pt = psA.tile([128, 128], BF16, tag="tps")
                nc.tensor.transpose(pt[:], src[:, blk * 128:(blk + 1) * 128], id_t[:])
                st = cp.tile([128, 128], BF16, tag=f"swemT{i}")
                nc.vector.tensor_copy(st[:], pt[:])
                swemT.append(st)

            # --- FC: h^T[mb] [128 hidden, 128 samples], mb in 0..7 ---
            h_ps = [psB.tile([128, 512], F32, tag="hps0", name="hps0"),
                    psB.tile([128, 512], F32, tag="hps1", name="hps1")]
            # PSUM `start` clears the has_written bits for the WHOLE bank, so
            # emit start only on the first matmul touching each bank and stop
            # only on the last matmul into that bank.
            for kc in range(16):
                w = wp.tile([128, H], BF16, tag="wfc")
                nc.sync.dma_start(w[:], wfc[kc * 128:(kc + 1) * 128, :])
                for mb in range(8):
                    nc.tensor.matmul(
                        h_ps[mb // 4][:, (mb % 4) * 128:(mb % 4 + 1) * 128],
                        lhsT=w[:, mb * 128:(mb + 1) * 128],
                        rhs=swemT[kc][:],
                        start=(kc == 0 and mb % 4 == 0),
                        stop=(kc == 15 and mb % 4 == 3))

            # --- batch stats: s1 = sum_n h, s2 = sum_n h^2 (per hidden) ---
            s12 = cp.tile([128, 16], F32, tag="s12")
            for mb in range(8):
                hps = h_ps[mb // 4][:, (mb % 4) * 128:(mb % 4 + 1) * 128]
                sq = sp.tile([128, 128], F32, tag="sq")
                nc.scalar.activation(sq[:], hps, AF.Square)
                nc.vector.reduce_sum(s12[:, mb:mb + 1], hps, axis=mybir.AxisListType.X)
                nc.vector.reduce_sum(s12[:, 8 + mb:9 + mb], sq[:], axis=mybir.AxisListType.X)

            # --- AllReduce batch stats across the 8 cores ---
            cc_in = dp.tile([128, 16], F32, tag="ccin")
            cc_out = dp.tile([128, 16], F32, tag="ccout")
            nc.sync.dma_start(cc_in[:], s12[:])
            nc.gpsimd.collective_compute(
                "AllReduce", OP.add,
                replica_groups=[list(range(N_CORES))],
                ins=[cc_in.opt()], outs=[cc_out.opt()],
            )
            s12g = cp.tile([128, 16], F32, tag="s12g")
            nc.sync.dma_start(s12g[:], cc_out[:])

            # --- BN scale/shift (per hidden unit, [128, 8]) ---
            mean = cp.tile([128, 8], F32, tag="mean")
            var = cp.tile([128, 8], F32, tag="var")
            scale = cp.tile([128, 8], F32, tag="scale")
            shift = cp.tile([128, 8], F32, tag="shift")
            inv_b = 1.0 / float(B)
            nc.vector.tensor_scalar_mul(mean[:], s12g[:, 0:8], inv_b)
            nc.vector.tensor_mul(scale[:], mean[:], mean[:])        # scale = mean^2 (tmp)
            nc.vector.tensor_scalar_mul(var[:], s12g[:, 8:16], inv_b)  # var = E[h^2]
            nc.vector.tensor_sub(var[:], var[:], scale[:])          # var -= mean^2
            nc.vector.tensor_scalar_add(var[:], var[:], BN_EPS)
            nc.scalar.activation(var[:], var[:], AF.Sqrt)           # var = std
            nc.vector.reciprocal(scale[:], var[:])                  # scale = 1/std
            nc.vector.tensor_mul(scale[:], scale[:], gam_t[:])      # scale *= gamma
            nc.vector.tensor_mul(shift[:], mean[:], scale[:])       # shift = mean*scale
            nc.vector.tensor_sub(shift[:], bet_t[:], shift[:])      # shift = beta - mean*scale

            # --- BN apply + ReLU on ACT, then classifier matmuls ---
            o_ps = psB.tile([128, C], F32, tag="ops")
            for mb in range(8):
                r = cp.tile([128, 128], BF16, tag=f"rT{mb}", name=f"rT{mb}")
                nc.scalar.activation(r[:], h_ps[mb // 4][:, (mb % 4) * 128:(mb % 4 + 1) * 128],
                                     AF.Relu,
                                     bias=shift[:, mb:mb + 1],
                                     scale=scale[:, mb:mb + 1])
                nc.tensor.matmul(o_ps[:], lhsT=r[:], rhs=wclf_t[mb][:],
                                 start=(mb == 0), stop=False)
            # + b_clf via rank-1 ones matmul
            nc.tensor.matmul(o_ps[:], lhsT=on_t[:], rhs=bc_t[:],
                             start=False, stop=True)
            out_sb = cp.tile([128, C], F32, tag="outsb")
            nc.vector.tensor_copy(out_sb[:], o_ps[:])
            nc.sync.dma_start(logits[:], out_sb[:])

    nc.compile()
    return nc


def _get_program():
    global _PROGRAM
    if _PROGRAM is None:
        _PROGRAM = _build()
    return _PROGRAM


def _wrap_idx(local, pos_list):
    """local: [PB, L] int16 local ids; returns wrapped [128, len*8] block."""
    cols = np.empty((16, len(pos_list) * 8), np.int16)
    flat = local[:, pos_list].T.reshape(-1)  # j-major: j*128 + p
    n = flat.shape[0]
    i = np.arange(n)
    cols[:, :] = 0
    cols[i % 16, i // 16] = flat
    return cols


def _prep(title, desc, t_len, d_len):
    title = np.asarray(title)
    desc = np.asarray(desc)
    t_len = np.asarray(t_len).astype(np.int64)
    d_len = np.asarray(d_len).astype(np.int64)

    # duplicate-index fill: invalid position p of row r takes the row's
    # chunk-start token if that is still valid, else token 0.
    def fill(tok, lens, L):
        pos = np.arange(L)[None, :]
        cs = (pos // KC) * KC
        chunk_start_tok = np.take_along_axis(
            tok, np.broadcast_to(cs, tok.shape), axis=1)
        dup = np.where(cs < lens[:, None], chunk_start_tok, tok[:, :1])
        return np.where(pos < lens[:, None], tok, dup).astype(np.int64)

    in_maps = []
    rng128 = np.arange(128)
    uniqs = []
    for i in range(N_CORES):
        sl = slice(i * PB, (i + 1) * PB)
        tl, dl = t_len[sl], d_len[sl]
        tt = fill(title[sl], tl, LT)
        dd = fill(desc[sl], dl, LD)
        allt = np.concatenate([tt, dd], axis=1)  # [PB, 250]
        uniq, inv = np.unique(allt, return_inverse=True)
        assert uniq.size <= U, uniq.size
        local = inv.reshape(PB, NPOS).astype(np.int16)
        uniqs.append(uniq)

        # idx blocks: per chunk, gathers of widths GW, flat j-major wrapped
        blocks = []
        for ci in range(NT + ND):
            base = ci * KC
            off = 0
            for w in GW:
                blocks.append(_wrap_idx(local, list(range(base + off,
                                                          base + off + w))))
                off += w
        idx16 = np.concatenate(blocks, axis=1)  # [16, 2000]
        idx_np = np.ascontiguousarray(np.tile(idx16, (8, 1)))

        scal = np.stack([
            1.0 / np.maximum(tl, 1.0),
            1.0 / np.maximum(dl, 1.0),
        ], axis=1).astype(np.float32)

        dn = np.zeros(((NT + ND) * 128, 128), dtype=BF_NP)
        ci = 0
        for lens, L in ((tl, LT), (dl, LD)):
            for c in range(L // KC):
                a, b_ = c * KC, (c + 1) * KC
                npad = np.clip(b_ - np.maximum(lens, a), 0, KC).astype(np.float32)
                dn[ci * 128 + rng128, rng128] = (-npad).astype(BF_NP)
                ci += 1

        in_maps.append({"idx": idx_np, "scal": scal, "dnpad": dn})
    return in_maps, uniqs


def kernel(title, desc, t_len, d_len, emb, W_fc, b_fc, gamma, beta,
           W_clf, b_clf):
    nc = _get_program()
    in_maps, uniqs = _prep(title, desc, t_len, d_len)

    emb_bf = np.asarray(emb, dtype=np.float32).astype(BF_NP)
    wfc_bf = np.ascontiguousarray(np.asarray(W_fc, dtype=np.float32).astype(BF_NP))
    wclf_bf = np.ascontiguousarray(np.asarray(W_clf, dtype=np.float32).astype(BF_NP))
    bclf_bf = np.asarray(b_clf, dtype=np.float32).astype(BF_NP).reshape(1, C)
    gamma_t = np.ascontiguousarray(
        np.asarray(gamma, dtype=np.float32).reshape(8, 128).T)
    beta_t = np.ascontiguousarray(
        np.asarray(beta, dtype=np.float32).reshape(8, 128).T)
    ident = np.eye(128, dtype=np.float32).astype(BF_NP)
    ones1 = np.ones((1, 128), dtype=np.float32).astype(BF_NP)

    for i, m in enumerate(in_maps):
        emb_local = np.zeros((U, D), dtype=BF_NP)
        emb_local[:uniqs[i].size] = emb_bf[uniqs[i]]
        m.update({"emb": emb_local, "wfc": wfc_bf, "wclf": wclf_bf,
                  "bclf": bclf_bf, "gamma_t": gamma_t, "beta_t": beta_t,
                  "ident": ident, "ones1": ones1})

    res = bass_utils.run_bass_kernel_spmd(nc, in_maps,
                                          core_ids=list(range(N_CORES)))
    return np.concatenate([np.asarray(res.results[i]["logits"])
                           for i in range(N_CORES)], axis=0)
